# revision 1
# baseline (speedup 1.0000x reference)
"""CBAM block kernel for Trainium2, 8-core data-parallel.

Computation (per image, C=256 channels, HW=56*56=3136 pixels):
  channel attention: spatial avg/max pool -> tiny MLP (BN+tanh) -> sigmoid -> ca[C]
  spatial attention: channel mean/max of ca*x -> reflect-pad 3x3 conv (2->1 ch)
                     -> two folded BNs -> sigmoid -> sa[HW]
  out = relu(fbn_scale * (x*ca*sa + x) + fbn_bias)
      = relu(x (*) M + bfb),  M = (sf*ca) (x) sa + sf (x) 1   (rank-2, via PE)

Sharding: pure data parallel, 4 images per NeuronCore, params replicated.

Key device techniques:
  - spatial sum/max pools: DVE tensor_scalar accum_out (op1=add / op1=max) at 2x
    rate, fused with the f32->bf16 cast of x
  - MLP runs channels-on-partitions so BN scale/bias are per-partition ACT args
  - channel max of ca*x: PE matmuls  x_bf16_block^T @ diag(ca_bf16)  transpose
    112-pixel blocks into PSUM supertiles, DVE reduce_max over the channel
    (free) axis; diag(ca) built by gpsimd affine_select
  - channel sum of ca*x: PE matmul with lhsT=ca (float32r fast path)
  - 3x3 conv: 3 PE matmuls against host-im2col'd banded weight matrices
    (both BNs + conv bias + 1/C mean factor folded in on host)
  - final: M from a K=2 outer-product matmul (float32r), DVE hadamard, relu
    with per-channel bias on ACT/DVE
"""

import os
from contextlib import ExitStack

import numpy as np

import concourse.bacc as bacc
import concourse.bass as bass
import concourse.mybir as mybir
import concourse.tile as tile
from concourse import bass_utils

F32 = mybir.dt.float32
F32R = mybir.dt.float32r
BF16 = mybir.dt.bfloat16
Alu = mybir.AluOpType
Act = mybir.ActivationFunctionType
AxX = mybir.AxisListType.X

B, C, H, W = 32, 256, 56, 56
HW = H * W                      # 3136
NCORES = 8
BLOC = B // NCORES              # 4 images per core
NCH = 2                         # channel chunks of 128
MID = C // 16                   # 16
GP = 112                        # pixels per transpose block (2 rows)
NGRP = HW // GP                 # 28
SUPG = 2                        # groups per psum supertile
NSUP = NGRP // SUPG             # 7
PIECE = 392                     # free-dim piece for the M/hadamard stage
NPIECE = HW // PIECE            # 8
OUTW = 1568                     # batched output store width (4 pieces)
APIECE = 448                    # avg-path psum piece
NAPIECE = HW // APIECE          # 7


# ---------------------------------------------------------------------------
# device program
# ---------------------------------------------------------------------------

def _build_program(loop_k=None):
    nc = bacc.Bacc(
        "TRN2",
        target_bir_lowering=False,
        debug=False,
        enable_asserts=False,
        num_devices=NCORES,
    )

    x_d = nc.dram_tensor("x_shard", [BLOC, C, HW], F32, kind="ExternalInput").ap()
    y_d = nc.dram_tensor("y_shard", [BLOC, C, HW], F32, kind="ExternalOutput").ap()
    w1a_d = nc.dram_tensor("w1t_avg", [C, MID], F32, kind="ExternalInput").ap()
    w1m_d = nc.dram_tensor("w1t_max", [C, MID], F32, kind="ExternalInput").ap()
    w2t_d = nc.dram_tensor("w2t", [MID, C], F32, kind="ExternalInput").ap()
    mlpv_d = nc.dram_tensor("mlp_vec", [MID, 2], F32, kind="ExternalInput").ap()
    chv_d = nc.dram_tensor("ch_vec", [C, 4], F32, kind="ExternalInput").ap()
    sfrow_d = nc.dram_tensor("sf_rows", [NCH, 128], BF16, kind="ExternalInput").ap()
    bmat_d = nc.dram_tensor("bmat", [116, 168], BF16, kind="ExternalInput").ap()
    ones_d = nc.dram_tensor("ones_row", [1, HW], BF16, kind="ExternalInput").ap()
    cst_d = nc.dram_tensor("conv_cst", [1, 1], F32, kind="ExternalInput").ap()

    with tile.TileContext(nc) as tc:
        with ExitStack() as ctx:
            if loop_k:
                with tc.For_i(0, loop_k, 1):
                    _trace_kernel(ctx, tc, y_d, x_d, w1a_d, w1m_d, w2t_d,
                                  mlpv_d, chv_d, sfrow_d, bmat_d, ones_d, cst_d)
            else:
                _trace_kernel(ctx, tc, y_d, x_d, w1a_d, w1m_d, w2t_d, mlpv_d,
                              chv_d, sfrow_d, bmat_d, ones_d, cst_d)
    nc.compile()
    return nc


def _trace_kernel(ctx, tc, y_d, x_d, w1a_d, w1m_d, w2t_d, mlpv_d, chv_d,
                  sfrow_d, bmat_d, ones_d, cst_d):
    nc = tc.nc

    consts = ctx.enter_context(tc.tile_pool(name="consts", bufs=1))
    px = ctx.enter_context(tc.tile_pool(name="px", bufs=2 * BLOC))
    pxb = ctx.enter_context(tc.tile_pool(name="pxb", bufs=6))
    pstat = ctx.enter_context(tc.tile_pool(name="pstat", bufs=16))
    pdiag = ctx.enter_context(tc.tile_pool(name="pdiag", bufs=8))
    prow = ctx.enter_context(tc.tile_pool(name="prow", bufs=2))
    pout = ctx.enter_context(tc.tile_pool(name="pout", bufs=4))
    ps_xt = ctx.enter_context(tc.tile_pool(name="ps_xt", bufs=2, space="PSUM"))
    ps_m = ctx.enter_context(tc.tile_pool(name="ps_m", bufs=2, space="PSUM"))
    ps_sm = ctx.enter_context(tc.tile_pool(name="ps_sm", bufs=2, space="PSUM"))
    ps_mlp = ctx.enter_context(tc.tile_pool(name="ps_mlp", bufs=2, space="PSUM"))

    # ---- constants into SBUF ----
    w1a = [consts.tile([128, MID], F32, tag=f"w1a{c}", name=f"w1a{c}") for c in range(NCH)]
    w1m = [consts.tile([128, MID], F32, tag=f"w1m{c}", name=f"w1m{c}") for c in range(NCH)]
    for c in range(NCH):
        nc.scalar.dma_start(out=w1a[c], in_=w1a_d[c * 128:(c + 1) * 128, :])
        nc.scalar.dma_start(out=w1m[c], in_=w1m_d[c * 128:(c + 1) * 128, :])
    w2t = consts.tile([MID, C], F32, tag="w2t")
    nc.scalar.dma_start(out=w2t, in_=w2t_d)
    mlpv = consts.tile([MID, 2], F32, tag="mlpv")
    nc.scalar.dma_start(out=mlpv, in_=mlpv_d)
    chv = [consts.tile([128, 4], F32, tag=f"chv{c}", name=f"chv{c}") for c in range(NCH)]
    for c in range(NCH):
        nc.scalar.dma_start(out=chv[c], in_=chv_d[c * 128:(c + 1) * 128, :])
    bmat = consts.tile([116, 168], BF16, tag="bmat")
    nc.scalar.dma_start(out=bmat, in_=bmat_d)
    cst56 = consts.tile([56, 1], F32, tag="cst56")
    nc.scalar.dma_start(out=cst56, in_=cst_d.to_broadcast((56, 1)))

    ones1 = consts.tile([128, 1], F32, tag="ones1")
    nc.vector.memset(ones1, 1.0)
    ident = consts.tile([128, 128], F32, tag="ident")
    nc.gpsimd.affine_select(
        out=ident, in_=ones1.broadcast_to((128, 128)), pattern=[[-1, 128]],
        compare_op=Alu.is_equal, fill=0.0, base=0, channel_multiplier=1,
    )

    PAIR = 2 if BLOC % 2 == 0 else 1
    NPAIR = BLOC // PAIR
    pend_finals = []

    def emit_final(i, xc, rhs2, lhsT2):
        for c in range(NCH):
            for half in range(HW // OUTW):
                ot = pout.tile([128, OUTW], F32, tag="ot", bufs=3, name="ot")
                for pp in range(OUTW // PIECE):
                    base = half * OUTW + pp * PIECE
                    sl = slice(base, base + PIECE)
                    osl = slice(pp * PIECE, (pp + 1) * PIECE)
                    mp = ps_m.tile([128, PIECE], F32, tag="mp", name="mp")
                    nc.tensor.matmul(out=mp, lhsT=lhsT2[c], rhs=rhs2[:, sl],
                                     start=True, stop=True)
                    nc.vector.tensor_tensor(out=ot[:, osl], in0=xc[c][:, sl],
                                            in1=mp, op=Alu.mult)
                nc.scalar.activation(out=ot, in_=ot, func=Act.Relu,
                                     bias=chv[c][:, 3:4], scale=1.0)
                nc.sync.dma_start(
                    out=y_d[i, c * 128:(c + 1) * 128,
                            half * OUTW:(half + 1) * OUTW], in_=ot)

    for pair in range(NPAIR):
        imgs = [pair * PAIR + j for j in range(PAIR)]

        # ---- load + pools + bf16 cast (both images of the pair) ----
        xcs, xbs = {}, {}
        ssum_all = [pstat.tile([128, PAIR], F32, tag=f"ssum{c}", name=f"ssum{c}")
                    for c in range(NCH)]
        smax_all = [pstat.tile([128, PAIR], F32, tag=f"smax{c}", name=f"smax{c}")
                    for c in range(NCH)]
        for j, i in enumerate(imgs):
            for c in range(NCH):
                xt = px.tile([128, HW], F32, tag="x")
                nc.sync.dma_start(out=xt, in_=x_d[i, c * 128:(c + 1) * 128, :])
                xcs[(j, c)] = xt
                xbt = pxb.tile([128, HW], BF16, tag="xb")
                nc.vector.tensor_scalar(out=xbt, in0=xt, scalar1=1.0,
                                        scalar2=None, op0=Alu.mult, op1=Alu.add,
                                        accum_out=ssum_all[c][:, j:j + 1])
                xtrash = pxb.tile([128, HW], BF16, tag="xbtrash", bufs=1)
                nc.vector.tensor_scalar(out=xtrash, in0=xbt, scalar1=1.0,
                                        scalar2=None, op0=Alu.mult, op1=Alu.max,
                                        accum_out=smax_all[c][:, j:j + 1])
                xbs[(j, c)] = xbt

        # ---- batched channel-attention MLP over the pair ----
        havg = ps_mlp.tile([MID, PAIR], F32, tag="mlp")
        hmax = ps_mlp.tile([MID, PAIR], F32, tag="mlp")
        for c in range(NCH):
            nc.tensor.matmul(out=havg, lhsT=w1a[c], rhs=ssum_all[c],
                             start=(c == 0), stop=(c == 1))
        for c in range(NCH):
            nc.tensor.matmul(out=hmax, lhsT=w1m[c], rhs=smax_all[c],
                             start=(c == 0), stop=(c == 1))
        tha = pstat.tile([MID, PAIR], F32, tag="tha")
        thm = pstat.tile([MID, PAIR], F32, tag="thm")
        nc.scalar.activation(out=tha, in_=havg, func=Act.Tanh,
                             bias=mlpv[:, 1:2], scale=mlpv[:, 0:1])
        nc.scalar.activation(out=thm, in_=hmax, func=Act.Tanh,
                             bias=mlpv[:, 1:2], scale=mlpv[:, 0:1])

        ca_all, cab_all, sfcaT_all = [], [], []
        for c in range(NCH):
            oa = ps_mlp.tile([128, PAIR], F32, tag="mlp")
            om = ps_mlp.tile([128, PAIR], F32, tag="mlp")
            nc.tensor.matmul(out=oa, lhsT=w2t[:, c * 128:(c + 1) * 128],
                             rhs=tha, start=True, stop=True)
            nc.tensor.matmul(out=om, lhsT=w2t[:, c * 128:(c + 1) * 128],
                             rhs=thm, start=True, stop=True)
            ta = pstat.tile([128, PAIR], F32, tag="ta")
            tm = pstat.tile([128, PAIR], F32, tag="tm")
            nc.scalar.activation(out=ta, in_=oa, func=Act.Tanh,
                                 bias=chv[c][:, 1:2], scale=chv[c][:, 0:1])
            nc.scalar.activation(out=tm, in_=om, func=Act.Tanh,
                                 bias=chv[c][:, 1:2], scale=chv[c][:, 0:1])
            sab = pstat.tile([128, PAIR], F32, tag="sab")
            nc.vector.tensor_tensor(out=sab, in0=ta, in1=tm, op=Alu.add)
            cat = pstat.tile([128, PAIR], F32, tag="ca")
            nc.scalar.activation(out=cat, in_=sab, func=Act.Sigmoid)
            ca_all.append(cat)
            cab = pstat.tile([128, PAIR], BF16, tag="cab")
            nc.vector.tensor_copy(out=cab, in_=cat)
            cab_all.append(cab)
            sfca = pstat.tile([128, PAIR], F32, tag="sfca")
            nc.vector.tensor_scalar(out=sfca, in0=cat, scalar1=chv[c][:, 2:3],
                                    scalar2=None, op0=Alu.mult)
            sfcaT_p = ps_mlp.tile([PAIR, 128], F32, tag="mlp")
            nc.tensor.transpose(out=sfcaT_p, in_=sfca, identity=ident)
            sfcaT = pstat.tile([PAIR, 128], BF16, tag="sfcaT")
            nc.vector.tensor_copy(out=sfcaT, in_=sfcaT_p)
            sfcaT_all.append(sfcaT)

        # ---- per-image spatial attention (both images) ----
        rhs2_all, lhsT2_all = {}, {}
        for j, i in enumerate(imgs):
            xc = [xcs[(j, c)] for c in range(NCH)]
            xb = [xbs[(j, c)] for c in range(NCH)]

            diag, lhsT2 = [], []
            for c in range(NCH):
                dg = pdiag.tile([128, 128], BF16, tag="diag")
                nc.gpsimd.affine_select(
                    out=dg, in_=ca_all[c][:, j:j + 1].broadcast_to((128, 128)),
                    pattern=[[-1, 128]], compare_op=Alu.is_equal,
                    fill=0.0, base=0, channel_multiplier=1,
                )
                diag.append(dg)
                l2 = prow.tile([2, 128], BF16, tag="lhsT2", bufs=8)
                nc.scalar.dma_start(out=l2[0:1, :], in_=sfcaT_all[c][j:j + 1, :])
                nc.scalar.dma_start(out=l2[1:2, :], in_=sfrow_d[c:c + 1, :])
                lhsT2.append(l2)

            # channel sum of ca*x (emit early: feeds conv via avg path)
            avg_row = prow.tile([1, HW], BF16, tag="avg_row")
            for p in range(NAPIECE):
                sl = slice(p * APIECE, (p + 1) * APIECE)
                ap = ps_sm.tile([1, APIECE], F32, tag="sm")
                for c in range(NCH):
                    nc.tensor.matmul(out=ap, lhsT=cab_all[c][:, j:j + 1],
                                     rhs=xb[c][:, sl],
                                     start=(c == 0), stop=(c == 1))
                nc.scalar.activation(out=avg_row[0:1, sl], in_=ap, func=Act.Copy)

            # channel max of ca*x (transpose blocks via diag matmul)
            mx = prow.tile([GP, NGRP], F32, tag="mx")
            for s in range(NSUP):
                sup = ps_xt.tile([GP, SUPG, C], F32, tag="sup")
                for gg in range(SUPG):
                    g = s * SUPG + gg
                    for c in range(NCH):
                        nc.tensor.matmul(
                            out=sup[:, gg, c * 128:(c + 1) * 128],
                            lhsT=xb[c][:, g * GP:(g + 1) * GP], rhs=diag[c],
                            start=True, stop=True,
                        )
                nc.vector.tensor_reduce(out=mx[:, s * SUPG:(s + 1) * SUPG],
                                        in_=sup, axis=AxX, op=Alu.max)
            mxT_p = ps_sm.tile([NGRP, GP], F32, tag="sm")
            nc.tensor.transpose(out=mxT_p, in_=mx, identity=ident[0:GP, 0:GP])
            mxT = prow.tile([NGRP, GP], BF16, tag="mxT")
            nc.vector.tensor_copy(out=mxT, in_=mxT_p)

            # reflect-padded conv input S = [avg(58) ; max(58)]
            S = prow.tile([116, 58], BF16, tag="S")
            nc.scalar.dma_start(out=S[1:57, 1:57], in_=avg_row)
            nc.scalar.dma_start(out=S[59:115, 1:57], in_=mxT)
            # reflect rows sourced from the flat rows (independent of interior)
            nc.scalar.dma_start(out=S[0:1, 1:57], in_=avg_row[0:1, 56:112])
            nc.scalar.dma_start(out=S[57:58, 1:57],
                                in_=avg_row[0:1, 54 * 56:55 * 56])
            nc.scalar.dma_start(out=S[58:59, 1:57], in_=mxT[0:1, 56:112])
            nc.scalar.dma_start(out=S[115:116, 1:57], in_=mxT[27:28, 0:56])
            nc.scalar.dma_start(out=S[:, 0:1], in_=S[:, 2:3])
            nc.scalar.dma_start(out=S[:, 57:58], in_=S[:, 55:56])

            # 3x3 conv as 3 banded matmuls + sigmoid -> sa
            conv = ps_sm.tile([56, 56], F32, tag="sm")
            for dx in range(3):
                nc.tensor.matmul(out=conv, lhsT=bmat[:, dx * 56:(dx + 1) * 56],
                                 rhs=S[:, dx:dx + 56],
                                 start=(dx == 0), stop=(dx == 2))
            sa56 = prow.tile([56, 56], BF16, tag="sa56")
            nc.scalar.activation(out=sa56, in_=conv, func=Act.Sigmoid,
                                 bias=cst56, scale=1.0)

            rhs2 = prow.tile([2, HW], BF16, tag="rhs2", bufs=3)
            nc.scalar.dma_start(out=rhs2[0:1, :], in_=sa56)
            nc.scalar.dma_start(out=rhs2[1:2, :], in_=ones_d)

            rhs2_all[j] = rhs2
            lhsT2_all[j] = lhsT2

        # emit this pair's finals (after both spatial chains)
        for j in range(PAIR):
            emit_final(imgs[j], [xcs[(j, c)] for c in range(NCH)],
                       rhs2_all[j], lhsT2_all[j])

    for args in pend_finals:
        emit_final(*args)


# ---------------------------------------------------------------------------
# host-side parameter folding
# ---------------------------------------------------------------------------

def _fold_params(inp):
    f = lambda a: np.asarray(a, dtype=np.float32)
    import ml_dtypes

    s1 = f(inp["bn1_g"]) / np.sqrt(f(inp["bn1_v"]) + 1e-5)
    b1 = f(inp["bn1_b"]) - f(inp["bn1_m"]) * s1
    s2 = f(inp["bn2_g"]) / np.sqrt(f(inp["bn2_v"]) + 1e-5)
    b2 = f(inp["bn2_b"]) - f(inp["bn2_m"]) * s2
    sf = f(inp["fbn_g"]) / np.sqrt(f(inp["fbn_v"]) + 1e-5)
    bfb = f(inp["fbn_b"]) - f(inp["fbn_m"]) * sf

    w1 = f(inp["w1"])                      # [MID, C]
    w2 = f(inp["w2"])                      # [C, MID]
    w1t_avg = np.ascontiguousarray((w1 / HW).T)     # [C, MID]
    w1t_max = np.ascontiguousarray(w1.T)            # [C, MID]
    w2t = np.ascontiguousarray(w2.T)                # [MID, C]
    mlp_vec = np.stack([s1, b1], axis=1)            # [MID, 2]
    ch_vec = np.stack([s2, b2, sf, bfb], axis=1)    # [C, 4]
    sf_rows = np.ascontiguousarray(sf.reshape(NCH, 128).astype(ml_dtypes.bfloat16))

    # spatial conv folding: two BNs + conv bias + channel-mean divisor
    a1 = f(inp["sbn1_g"])[0] / np.sqrt(f(inp["sbn1_v"])[0] + 1e-3)
    c1 = f(inp["sbn1_b"])[0] - f(inp["sbn1_m"])[0] * a1
    a2 = f(inp["sbn2_g"])[0] / np.sqrt(f(inp["sbn2_v"])[0] + 1e-5)
    c2 = f(inp["sbn2_b"])[0] - f(inp["sbn2_m"])[0] * a2
    amul = a1 * a2
    cst = a2 * (a1 * f(inp["sconv_b"])[0] + c1) + c2

    wsp = f(inp["sconv_w"])[0]             # [2, 3, 3]
    w_eff = np.stack([wsp[0] * amul / C, wsp[1] * amul])  # [2(ic), 3(dy), 3(dx)]

    bmat = np.zeros((116, 168), np.float32)   # [ic*58+yp, dx*56+y]
    for ic in range(2):
        for dx in range(3):
            for y in range(56):
                for dy in range(3):
                    bmat[ic * 58 + y + dy, dx * 56 + y] = w_eff[ic, dy, dx]

    return {
        "w1t_avg": w1t_avg, "w1t_max": w1t_max, "w2t": w2t,
        "mlp_vec": np.ascontiguousarray(mlp_vec),
        "ch_vec": np.ascontiguousarray(ch_vec),
        "sf_rows": sf_rows, "bmat": bmat.astype(ml_dtypes.bfloat16),
        "ones_row": np.ones((1, HW), ml_dtypes.bfloat16),
        "conv_cst": np.full((1, 1), cst, np.float32),
    }


_NC_CACHE = {}


def _get_program():
    if "nc" not in _NC_CACHE:
        _NC_CACHE["nc"] = _build_program()
    return _NC_CACHE["nc"]


def kernel(**inputs) -> np.ndarray:
    nc = _get_program()
    params = _fold_params(inputs)
    x = np.asarray(inputs["x"], dtype=np.float32).reshape(B, C, HW)

    in_maps = []
    for core in range(NCORES):
        shard = np.ascontiguousarray(x[core * BLOC:(core + 1) * BLOC])
        in_maps.append({"x_shard": shard, **params})

    res = bass_utils.run_bass_kernel_spmd(nc, in_maps, core_ids=list(range(NCORES)))
    out = np.concatenate([r["y_shard"] for r in res.results], axis=0)
    return out.reshape(B, C, H, W).astype(np.float32)



# revision 39
# speedup vs baseline: 10.5746x; 10.5746x over previous
"""CBAM block kernel for Trainium2, 8-core data-parallel (v5).

Computation (per image, C=256 channels, HW=56*56=3136 pixels):
  channel attention: spatial avg/max pool -> tiny MLP (BN+tanh) -> sigmoid -> ca[C]
  spatial attention: channel mean/max of ca*x -> reflect-pad 3x3 conv (2->1 ch)
                     -> two folded BNs -> sigmoid -> sa[HW]
  out = relu(fbn_scale * (x*ca*sa + x) + fbn_bias)
      = relu(x * M + bfb),  M = (sf*ca) (x) sa + sf (x) 1

Design (everything bf16 in SBUF, f32 only for stats/scalars):
  - host converts x to bf16; y returned bf16 and upcast on host (halves HBM)
  - sum pool on ACT (activation Copy + accum_out), max pool on DVE
    tensor_scalar at 4x bf16 rate
  - channel max of ca*x: PE transposes xb*diag(ca) into PSUM supertiles
    (diag via gpsimd affine_select), DVE tensor_reduce(max), PE transpose
    back, ACT copy to bf16
  - channel sum via PE matmuls with ca lhsT, PSUM pieces packed at
    partition bases {0,32,64}, one ACT copy per packed tile
  - sa row bounced through DRAM scratch, then one broadcast DMA -> saB
  - M = sfca*saB + sf by one DVE tensor_scalar (4x, two scalar APs)
  - final: prod = TT(xb, M) at 2x on DVE; relu+bias on ACT; DMA out bf16
  - software-pipelined emission: loads first, pools staged two images
    ahead, per-image MLP, finals for all images at the end
"""

import os
from contextlib import ExitStack

import numpy as np

import concourse.bacc as bacc
import concourse.bass as bass
import concourse.mybir as mybir
import concourse.tile as tile
from concourse import bass_utils

F32 = mybir.dt.float32
BF16 = mybir.dt.bfloat16
Alu = mybir.AluOpType
Act = mybir.ActivationFunctionType
AxX = mybir.AxisListType.X

B, C, H, W = 32, 256, 56, 56
HW = H * W                      # 3136
NCORES = 8
BLOC = B // NCORES              # 4 images per core
NCH = 2                         # channel chunks of 128
MID = C // 16                   # 16
GP = 112                        # pixels per transpose group (2 rows)
NGRP = HW // GP                 # 28
SUPG = 4                        # groups per psum supertile (2 banks)
NSUP = NGRP // SUPG             # 7
APIECE = 448                    # avg-path psum piece
NAPIECE = HW // APIECE          # 7


def _build_program(loop_k=None):
    nc = bacc.Bacc(
        "TRN2",
        target_bir_lowering=False,
        debug=False,
        enable_asserts=False,
        num_devices=NCORES,
    )

    x_d = nc.dram_tensor("x_shard", [BLOC, C, HW], BF16, kind="ExternalInput").ap()
    y_d = nc.dram_tensor("y_shard", [BLOC, C, HW], BF16, kind="ExternalOutput").ap()
    w1a_d = nc.dram_tensor("w1t_avg", [C, MID], F32, kind="ExternalInput").ap()
    w1m_d = nc.dram_tensor("w1t_max", [C, MID], F32, kind="ExternalInput").ap()
    w2t_d = nc.dram_tensor("w2t", [MID, C], F32, kind="ExternalInput").ap()
    mlpv_d = nc.dram_tensor("mlp_vec", [MID, 2], F32, kind="ExternalInput").ap()
    chv_d = nc.dram_tensor("ch_vec", [C, 4], F32, kind="ExternalInput").ap()
    bmat_d = nc.dram_tensor("bmat", [116, 168], BF16, kind="ExternalInput").ap()
    cst_d = nc.dram_tensor("conv_cst", [1, 1], F32, kind="ExternalInput").ap()
    scr_d = nc.dram_tensor("sa_scratch", [BLOC, HW], BF16, kind="Internal").ap()

    with tile.TileContext(nc) as tc:
        with ExitStack() as ctx:
            if loop_k:
                with tc.For_i(0, loop_k, 1):
                    _trace_kernel(ctx, tc, y_d, x_d, w1a_d, w1m_d, w2t_d,
                                  mlpv_d, chv_d, bmat_d, cst_d, scr_d)
            else:
                _trace_kernel(ctx, tc, y_d, x_d, w1a_d, w1m_d, w2t_d, mlpv_d,
                              chv_d, bmat_d, cst_d, scr_d)
    nc.compile()
    return nc


def _trace_kernel(ctx, tc, y_d, x_d, w1a_d, w1m_d, w2t_d, mlpv_d, chv_d,
                  bmat_d, cst_d, scr_d):
    nc = tc.nc

    consts = ctx.enter_context(tc.tile_pool(name="consts", bufs=1))
    pxb = ctx.enter_context(tc.tile_pool(name="pxb", bufs=2 * BLOC))
    ptrash = ctx.enter_context(tc.tile_pool(name="ptrash", bufs=2))
    pstat = ctx.enter_context(tc.tile_pool(name="pstat", bufs=16))
    pdiag = ctx.enter_context(tc.tile_pool(name="pdiag", bufs=4))
    pbig = ctx.enter_context(tc.tile_pool(name="pbig", bufs=3))
    psmall = ctx.enter_context(tc.tile_pool(name="psmall", bufs=2))
    ps_xt = ctx.enter_context(tc.tile_pool(name="ps_xt", bufs=2, space="PSUM"))
    ps_mlp = ctx.enter_context(tc.tile_pool(name="ps_mlp", bufs=2, space="PSUM"))
    ps_avg = ctx.enter_context(tc.tile_pool(name="ps_avg", bufs=1, space="PSUM"))
    ps_sm = ctx.enter_context(tc.tile_pool(name="ps_sm", bufs=1, space="PSUM"))

    # ---- constants into SBUF ----
    w1a = [consts.tile([128, MID], F32, tag=f"w1a{c}", name=f"w1a{c}") for c in range(NCH)]
    w1m = [consts.tile([128, MID], F32, tag=f"w1m{c}", name=f"w1m{c}") for c in range(NCH)]
    for c in range(NCH):
        nc.scalar.dma_start(out=w1a[c], in_=w1a_d[c * 128:(c + 1) * 128, :])
        nc.scalar.dma_start(out=w1m[c], in_=w1m_d[c * 128:(c + 1) * 128, :])
    w2t = consts.tile([MID, C], F32, tag="w2t")
    nc.scalar.dma_start(out=w2t, in_=w2t_d)
    mlpv = consts.tile([MID, 2], F32, tag="mlpv")
    nc.scalar.dma_start(out=mlpv, in_=mlpv_d)
    chv = [consts.tile([128, 4], F32, tag=f"chv{c}", name=f"chv{c}") for c in range(NCH)]
    for c in range(NCH):
        nc.scalar.dma_start(out=chv[c], in_=chv_d[c * 128:(c + 1) * 128, :])
    bmat = consts.tile([116, 168], BF16, tag="bmat")
    nc.scalar.dma_start(out=bmat, in_=bmat_d)
    cst56 = consts.tile([56, 1], F32, tag="cst56")
    nc.scalar.dma_start(out=cst56, in_=cst_d.to_broadcast((56, 1)))
    onesf = consts.tile([128, 1], F32, tag="onesf")
    nc.vector.memset(onesf, 1.0)
    identF = consts.tile([128, 128], F32, tag="identF")
    nc.gpsimd.affine_select(
        out=identF, in_=onesf.broadcast_to((128, 128)), pattern=[[-1, 128]],
        compare_op=Alu.is_equal, fill=0.0, base=0, channel_multiplier=1,
    )

    # ---- all loads up front, split across SP and ACT queues ----
    xbs = {}
    for i in range(BLOC):
        for c in range(NCH):
            xbt = pxb.tile([128, HW], BF16, tag="xb")
            q = nc.sync if (i * NCH + c) % 2 == 0 else nc.scalar
            q.dma_start(out=xbt, in_=x_d[i, c * 128:(c + 1) * 128, :])
            xbs[(i, c)] = xbt

    def emit_pools(i, ssum, smax):
        for c in range(NCH):
            trA = ptrash.tile([128, HW], BF16, tag="trA")
            nc.scalar.activation(out=trA, in_=xbs[(i, c)], func=Act.Copy,
                                 accum_out=ssum[c])
            trB = ptrash.tile([128, HW], BF16, tag="trB")
            nc.vector.tensor_scalar(out=trB, in0=xbs[(i, c)], scalar1=1.0,
                                    scalar2=None, op0=Alu.mult, op1=Alu.max,
                                    accum_out=smax[c])

    def emit_mlp(ssum, smax):
        mlpt = ps_mlp.tile([128, 2], F32, tag="mlp", name="mlp1")
        mlp1 = mlpt[0:MID, :]
        for c in range(NCH):
            nc.tensor.matmul(out=mlp1[:, 0:1], lhsT=w1a[c], rhs=ssum[c],
                             start=(c == 0), stop=(c == 1))
        for c in range(NCH):
            nc.tensor.matmul(out=mlp1[:, 1:2], lhsT=w1m[c], rhs=smax[c],
                             start=(c == 0), stop=(c == 1))
        th1 = pstat.tile([MID, 2], F32, tag="th1")
        nc.scalar.activation(out=th1, in_=mlp1, func=Act.Tanh,
                             bias=mlpv[:, 1:2], scale=mlpv[:, 0:1])
        ca, cab, sfca = [], [], []
        for c in range(NCH):
            mlp2 = ps_mlp.tile([128, 2], F32, tag="mlp", name="mlp2")
            nc.tensor.matmul(out=mlp2[:, 0:1],
                             lhsT=w2t[:, c * 128:(c + 1) * 128],
                             rhs=th1[:, 0:1], start=True, stop=True)
            nc.tensor.matmul(out=mlp2[:, 1:2],
                             lhsT=w2t[:, c * 128:(c + 1) * 128],
                             rhs=th1[:, 1:2], start=True, stop=True)
            th2 = pstat.tile([128, 2], F32, tag="th2")
            nc.scalar.activation(out=th2, in_=mlp2, func=Act.Tanh,
                                 bias=chv[c][:, 1:2], scale=chv[c][:, 0:1])
            sab = pstat.tile([128, 1], F32, tag="sab")
            nc.vector.tensor_tensor(out=sab, in0=th2[:, 0:1],
                                    in1=th2[:, 1:2], op=Alu.add)
            cat = pstat.tile([128, 1], F32, tag="ca")
            nc.scalar.activation(out=cat, in_=sab, func=Act.Sigmoid)
            ca.append(cat)
            cb = pstat.tile([128, 1], BF16, tag="cab")
            nc.vector.tensor_copy(out=cb, in_=cat)
            cab.append(cb)
            sf = pstat.tile([128, 1], F32, tag="sfca")
            nc.vector.tensor_scalar(out=sf, in0=cat, scalar1=chv[c][:, 2:3],
                                    scalar2=None, op0=Alu.mult)
            sfca.append(sf)
        return ca, cab, sfca

    def emit_spatial(i, cab):
        xb = [xbs[(i, c)] for c in range(NCH)]

        diag = []
        for c in range(NCH):
            dg = pdiag.tile([128, 128], BF16, tag="diag")
            nc.gpsimd.affine_select(
                out=dg, in_=cab[c].broadcast_to((128, 128)),
                pattern=[[-1, 128]], compare_op=Alu.is_equal,
                fill=0.0, base=0, channel_multiplier=1,
            )
            diag.append(dg)

        # channel sum of ca*x -> psum pieces at partition bases {0,32,64}
        avgt = []
        for t in range(3):
            at = ps_avg.tile([65, APIECE], F32, tag="avg", name=f"avg{t}")
            avgt.append(at)
            for pp in range(3 if t < 2 else 1):
                p = t * 3 + pp
                sl = slice(p * APIECE, (p + 1) * APIECE)
                for c in range(NCH):
                    nc.tensor.matmul(out=at[pp * 32:pp * 32 + 1, :],
                                     lhsT=cab[c], rhs=xb[c][:, sl],
                                     start=(c == 0), stop=(c == 1))
        avgS = []
        for t in range(3):
            aS = psmall.tile([65, APIECE], BF16, tag=f"avgS{t}",
                             name=f"avgS{t}")
            nc.scalar.activation(out=aS, in_=avgt[t], func=Act.Copy)
            avgS.append(aS)

        # channel max of ca*x: transpose blocks via diag matmul + reduce
        mx = psmall.tile([GP, NGRP], F32, tag="mx")
        for s in range(NSUP):
            sup = ps_xt.tile([GP, SUPG, C], F32, tag="sup")
            for gg in range(SUPG):
                g = s * SUPG + gg
                for c in range(NCH):
                    nc.tensor.matmul(
                        out=sup[:, gg, c * 128:(c + 1) * 128],
                        lhsT=xb[c][:, g * GP:(g + 1) * GP], rhs=diag[c],
                        start=True, stop=True,
                    )
            nc.vector.tensor_reduce(out=mx[:, s * SUPG:(s + 1) * SUPG],
                                    in_=sup, axis=AxX, op=Alu.max)
        mxT_p = ps_sm.tile([56, APIECE], F32, tag="sm", name="mxTp")
        nc.tensor.transpose(out=mxT_p[0:NGRP, 0:GP], in_=mx,
                            identity=identF[0:GP, 0:GP])
        mxT = psmall.tile([NGRP, GP], BF16, tag="mxT")
        nc.scalar.activation(out=mxT, in_=mxT_p[0:NGRP, 0:GP], func=Act.Copy)

        # reflect-padded conv input S = [avg(58) ; max(58)]
        S = psmall.tile([116, 58], BF16, tag="S")
        for p in range(NAPIECE):
            nc.sync.dma_start(
                out=S[1 + 8 * p:9 + 8 * p, 1:57],
                in_=avgS[p // 3][(p % 3) * 32:(p % 3) * 32 + 1, :])
        nc.sync.dma_start(out=S[59:115, 1:57], in_=mxT)
        nc.sync.dma_start(out=S[0:1, 1:57], in_=avgS[0][0:1, 56:112])
        nc.sync.dma_start(out=S[57:58, 1:57], in_=avgS[2][0:1, 336:392])
        nc.sync.dma_start(out=S[58:59, 1:57], in_=mxT[0:1, 56:112])
        nc.sync.dma_start(out=S[115:116, 1:57], in_=mxT[27:28, 0:56])
        nc.scalar.activation(out=S[:, 0:1], in_=S[:, 2:3], func=Act.Copy)
        nc.scalar.activation(out=S[:, 57:58], in_=S[:, 55:56], func=Act.Copy)

        # 3x3 conv as 3 banded matmuls + sigmoid -> sa
        conv = ps_sm.tile([56, APIECE], F32, tag="sm", name="conv")
        for dx in range(3):
            nc.tensor.matmul(out=conv[:, 0:56],
                             lhsT=bmat[:, dx * 56:(dx + 1) * 56],
                             rhs=S[:, dx:dx + 56],
                             start=(dx == 0), stop=(dx == 2))
        sa56 = psmall.tile([56, 56], BF16, tag="sa56")
        nc.scalar.activation(out=sa56, in_=conv[:, 0:56], func=Act.Sigmoid,
                             bias=cst56, scale=1.0)
        # bounce sa through DRAM, then broadcast to 128 partitions
        nc.sync.dma_start(out=scr_d[i:i + 1, :], in_=sa56)
        saB = pbig.tile([128, HW], BF16, tag="saB", bufs=2)
        nc.sync.dma_start(out=saB,
                          in_=scr_d[i:i + 1, :].to_broadcast((128, HW)))
        return saB

    # ---- software-pipelined front-end: pools staged two images ahead ----
    stats = {}
    for i in range(BLOC):
        stats[i] = ([pstat.tile([128, 1], F32, tag=f"ssum{c}{i % 2}",
                                name=f"ssum{c}_{i}") for c in range(NCH)],
                    [pstat.tile([128, 1], F32, tag=f"smax{c}{i % 2}",
                                name=f"smax{c}_{i}") for c in range(NCH)])

    def emit_finals(i):
        prods = {}
        for c in range(NCH):
            Mt = pbig.tile([128, HW], BF16, tag="M", bufs=3)
            nc.vector.tensor_scalar(out=Mt, in0=saB_all[i],
                                    scalar1=sfca_all[i][c],
                                    scalar2=chv[c][:, 2:3],
                                    op0=Alu.mult, op1=Alu.add)
            prod = pbig.tile([128, HW], BF16, tag="prod", bufs=4)
            nc.vector.tensor_tensor(out=prod, in0=xbs[(i, c)], in1=Mt,
                                    op=Alu.mult)
            prods[c] = prod
        for c in range(NCH):
            prod = prods[c]
            nc.scalar.activation(out=prod, in_=prod, func=Act.Relu,
                                 bias=chv[c][:, 3:4], scale=1.0)
            nc.sync.dma_start(
                out=y_d[i, c * 128:(c + 1) * 128, :], in_=prod)

    saB_all, sfca_all = {}, {}
    emit_pools(0, *stats[0])
    emit_pools(1, *stats[1])
    for i in range(BLOC):
        if i + 2 < BLOC:
            emit_pools(i + 2, *stats[i + 2])
        ca, cab, sfca = emit_mlp(*stats[i])
        sfca_all[i] = sfca
        saB_all[i] = emit_spatial(i, cab)
    for i in range(BLOC):
        emit_finals(i)


# ---------------------------------------------------------------------------
# host-side parameter folding
# ---------------------------------------------------------------------------

def _fold_params(inp):
    f = lambda a: np.asarray(a, dtype=np.float32)
    import ml_dtypes

    s1 = f(inp["bn1_g"]) / np.sqrt(f(inp["bn1_v"]) + 1e-5)
    b1 = f(inp["bn1_b"]) - f(inp["bn1_m"]) * s1
    s2 = f(inp["bn2_g"]) / np.sqrt(f(inp["bn2_v"]) + 1e-5)
    b2 = f(inp["bn2_b"]) - f(inp["bn2_m"]) * s2
    sf = f(inp["fbn_g"]) / np.sqrt(f(inp["fbn_v"]) + 1e-5)
    bfb = f(inp["fbn_b"]) - f(inp["fbn_m"]) * sf

    w1 = f(inp["w1"])                      # [MID, C]
    w2 = f(inp["w2"])                      # [C, MID]
    w1t_avg = np.ascontiguousarray((w1 / HW).T)     # [C, MID]
    w1t_max = np.ascontiguousarray(w1.T)            # [C, MID]
    w2t = np.ascontiguousarray(w2.T)                # [MID, C]
    mlp_vec = np.stack([s1, b1], axis=1)            # [MID, 2]
    ch_vec = np.stack([s2, b2, sf, bfb], axis=1)    # [C, 4]

    # spatial conv folding: two BNs + conv bias + channel-mean divisor
    a1 = f(inp["sbn1_g"])[0] / np.sqrt(f(inp["sbn1_v"])[0] + 1e-3)
    c1 = f(inp["sbn1_b"])[0] - f(inp["sbn1_m"])[0] * a1
    a2 = f(inp["sbn2_g"])[0] / np.sqrt(f(inp["sbn2_v"])[0] + 1e-5)
    c2 = f(inp["sbn2_b"])[0] - f(inp["sbn2_m"])[0] * a2
    amul = a1 * a2
    cst = a2 * (a1 * f(inp["sconv_b"])[0] + c1) + c2

    wsp = f(inp["sconv_w"])[0]             # [2, 3, 3]
    w_eff = np.stack([wsp[0] * amul / C, wsp[1] * amul])  # [2(ic), 3(dy), 3(dx)]

    bmat = np.zeros((116, 168), np.float32)   # [ic*58+yp, dx*56+y]
    for ic in range(2):
        for dx in range(3):
            for y in range(56):
                for dy in range(3):
                    bmat[ic * 58 + y + dy, dx * 56 + y] = w_eff[ic, dy, dx]

    return {
        "w1t_avg": w1t_avg, "w1t_max": w1t_max, "w2t": w2t,
        "mlp_vec": np.ascontiguousarray(mlp_vec),
        "ch_vec": np.ascontiguousarray(ch_vec),
        "bmat": bmat.astype(ml_dtypes.bfloat16),
        "conv_cst": np.full((1, 1), cst, np.float32),
    }


_NC_CACHE = {}


def _get_program():
    if "nc" not in _NC_CACHE:
        _NC_CACHE["nc"] = _build_program()
    return _NC_CACHE["nc"]


def make_in_maps(inputs):
    import ml_dtypes
    params = _fold_params(inputs)
    x = np.asarray(inputs["x"], dtype=np.float32).reshape(B, C, HW)
    xb = x.astype(ml_dtypes.bfloat16)
    return [{"x_shard": np.ascontiguousarray(xb[core * BLOC:(core + 1) * BLOC]),
             **params} for core in range(NCORES)]


def kernel(**inputs) -> np.ndarray:
    nc = _get_program()
    in_maps = make_in_maps(inputs)
    res = bass_utils.run_bass_kernel_spmd(nc, in_maps, core_ids=list(range(NCORES)))
    out = np.concatenate([r["y_shard"].astype(np.float32) for r in res.results],
                         axis=0)
    return out.reshape(B, C, H, W)


# revision 47
# speedup vs baseline: 10.5982x; 1.0022x over previous
"""CBAM block kernel for Trainium2, 8-core data-parallel (v5).

Computation (per image, C=256 channels, HW=56*56=3136 pixels):
  channel attention: spatial avg/max pool -> tiny MLP (BN+tanh) -> sigmoid -> ca[C]
  spatial attention: channel mean/max of ca*x -> reflect-pad 3x3 conv (2->1 ch)
                     -> two folded BNs -> sigmoid -> sa[HW]
  out = relu(fbn_scale * (x*ca*sa + x) + fbn_bias)
      = relu(x * M + bfb),  M = (sf*ca) (x) sa + sf (x) 1

Design (everything bf16 in SBUF, f32 only for stats/scalars):
  - host converts x to bf16; y returned bf16 and upcast on host (halves HBM)
  - sum pool on ACT (activation Copy + accum_out), max pool on DVE
    tensor_scalar at 4x bf16 rate
  - channel max of ca*x: PE transposes xb*diag(ca) into PSUM supertiles
    (diag via gpsimd affine_select), DVE tensor_reduce(max), PE transpose
    back, ACT copy to bf16
  - channel sum via PE matmuls with ca lhsT, PSUM pieces packed at
    partition bases {0,32,64}, one ACT copy per packed tile
  - sa row bounced through DRAM scratch, then one broadcast DMA -> saB
  - M = sfca*saB + sf by one DVE tensor_scalar (4x, two scalar APs)
  - final: prod = TT(xb, M) at 2x on DVE; relu+bias on ACT; DMA out bf16
  - software-pipelined emission: loads first, pools staged two images
    ahead, per-image MLP, finals for all images at the end
"""

import os
from contextlib import ExitStack

import numpy as np

import concourse.bacc as bacc
import concourse.bass as bass
import concourse.bass_isa as bass_isa
import concourse.mybir as mybir
import concourse.tile as tile
from concourse import bass_utils

F32 = mybir.dt.float32
BF16 = mybir.dt.bfloat16
Alu = mybir.AluOpType
Act = mybir.ActivationFunctionType
AxX = mybir.AxisListType.X

B, C, H, W = 32, 256, 56, 56
HW = H * W                      # 3136
NCORES = 8
BLOC = B // NCORES              # 4 images per core
NCH = 2                         # channel chunks of 128
MID = C // 16                   # 16
GP = 112                        # pixels per transpose group (2 rows)
NGRP = HW // GP                 # 28
SUPG = 4                        # groups per psum supertile (2 banks)
NSUP = NGRP // SUPG             # 7
APIECE = 448                    # avg-path psum piece
NAPIECE = HW // APIECE          # 7


def _build_program(loop_k=None):
    nc = bacc.Bacc(
        "TRN2",
        target_bir_lowering=False,
        debug=False,
        enable_asserts=False,
        num_devices=NCORES,
    )

    x_d = nc.dram_tensor("x_shard", [BLOC, C, HW], BF16, kind="ExternalInput").ap()
    y_d = nc.dram_tensor("y_shard", [BLOC, C, HW], BF16, kind="ExternalOutput").ap()
    w1a_d = nc.dram_tensor("w1t_avg", [C, MID], F32, kind="ExternalInput").ap()
    w1m_d = nc.dram_tensor("w1t_max", [C, MID], F32, kind="ExternalInput").ap()
    w2t_d = nc.dram_tensor("w2t", [MID, C], F32, kind="ExternalInput").ap()
    mlpv_d = nc.dram_tensor("mlp_vec", [MID, 2], F32, kind="ExternalInput").ap()
    chv_d = nc.dram_tensor("ch_vec", [C, 4], F32, kind="ExternalInput").ap()
    bmat_d = nc.dram_tensor("bmat", [116, 168], BF16, kind="ExternalInput").ap()
    cst_d = nc.dram_tensor("conv_cst", [1, 1], F32, kind="ExternalInput").ap()
    scr_d = nc.dram_tensor("sa_scratch", [BLOC, HW], BF16, kind="Internal").ap()

    with tile.TileContext(nc) as tc:
        with ExitStack() as ctx:
            if loop_k:
                with tc.For_i(0, loop_k, 1):
                    _trace_kernel(ctx, tc, y_d, x_d, w1a_d, w1m_d, w2t_d,
                                  mlpv_d, chv_d, bmat_d, cst_d, scr_d)
            else:
                _trace_kernel(ctx, tc, y_d, x_d, w1a_d, w1m_d, w2t_d, mlpv_d,
                              chv_d, bmat_d, cst_d, scr_d)
    nc.compile()
    return nc


def _trace_kernel(ctx, tc, y_d, x_d, w1a_d, w1m_d, w2t_d, mlpv_d, chv_d,
                  bmat_d, cst_d, scr_d):
    nc = tc.nc

    consts = ctx.enter_context(tc.tile_pool(name="consts", bufs=1))
    pxb = ctx.enter_context(tc.tile_pool(name="pxb", bufs=2 * BLOC))
    ptrash = ctx.enter_context(tc.tile_pool(name="ptrash", bufs=2))
    pstat = ctx.enter_context(tc.tile_pool(name="pstat", bufs=16))
    pdiag = ctx.enter_context(tc.tile_pool(name="pdiag", bufs=4))
    pr = ctx.enter_context(tc.tile_pool(name="pr", bufs=3))
    pbig = ctx.enter_context(tc.tile_pool(name="pbig", bufs=3))
    psmall = ctx.enter_context(tc.tile_pool(name="psmall", bufs=2))
    ps_xt = ctx.enter_context(tc.tile_pool(name="ps_xt", bufs=2, space="PSUM"))
    ps_mlp = ctx.enter_context(tc.tile_pool(name="ps_mlp", bufs=2, space="PSUM"))
    ps_avg = ctx.enter_context(tc.tile_pool(name="ps_avg", bufs=1, space="PSUM"))
    ps_sm = ctx.enter_context(tc.tile_pool(name="ps_sm", bufs=1, space="PSUM"))

    # ---- constants into SBUF ----
    w1a = [consts.tile([128, MID], F32, tag=f"w1a{c}", name=f"w1a{c}") for c in range(NCH)]
    w1m = [consts.tile([128, MID], F32, tag=f"w1m{c}", name=f"w1m{c}") for c in range(NCH)]
    for c in range(NCH):
        nc.scalar.dma_start(out=w1a[c], in_=w1a_d[c * 128:(c + 1) * 128, :])
        nc.scalar.dma_start(out=w1m[c], in_=w1m_d[c * 128:(c + 1) * 128, :])
    w2t = consts.tile([MID, C], F32, tag="w2t")
    nc.scalar.dma_start(out=w2t, in_=w2t_d)
    mlpv = consts.tile([MID, 2], F32, tag="mlpv")
    nc.scalar.dma_start(out=mlpv, in_=mlpv_d)
    chv = [consts.tile([128, 4], F32, tag=f"chv{c}", name=f"chv{c}") for c in range(NCH)]
    for c in range(NCH):
        nc.scalar.dma_start(out=chv[c], in_=chv_d[c * 128:(c + 1) * 128, :])
    bmat = consts.tile([116, 168], BF16, tag="bmat")
    nc.scalar.dma_start(out=bmat, in_=bmat_d)
    cst56 = consts.tile([56, 1], F32, tag="cst56")
    nc.scalar.dma_start(out=cst56, in_=cst_d.to_broadcast((56, 1)))
    onesf = consts.tile([128, 1], F32, tag="onesf")
    nc.vector.memset(onesf, 1.0)
    identF = consts.tile([128, 128], F32, tag="identF")
    nc.gpsimd.affine_select(
        out=identF, in_=onesf.broadcast_to((128, 128)), pattern=[[-1, 128]],
        compare_op=Alu.is_equal, fill=0.0, base=0, channel_multiplier=1,
    )

    # ---- all loads up front, split across SP and ACT queues ----
    xbs = {}
    for i in range(BLOC):
        for c in range(NCH):
            xbt = pxb.tile([128, HW], BF16, tag="xb")
            q = nc.sync if (i * NCH + c) % 2 == 0 else nc.scalar
            q.dma_start(out=xbt, in_=x_d[i, c * 128:(c + 1) * 128, :])
            xbs[(i, c)] = xbt

    def emit_pools(i, ssum, smax):
        for c in range(NCH):
            trA = ptrash.tile([128, HW], BF16, tag="trA")
            nc.scalar.activation(out=trA, in_=xbs[(i, c)], func=Act.Copy,
                                 accum_out=ssum[c])
            trB = ptrash.tile([128, HW], BF16, tag="trB")
            nc.vector.tensor_scalar(out=trB, in0=xbs[(i, c)], scalar1=1.0,
                                    scalar2=None, op0=Alu.mult, op1=Alu.max,
                                    accum_out=smax[c])

    def emit_mlp(ssum, smax):
        mlpt = ps_mlp.tile([128, 2], F32, tag="mlp", name="mlp1")
        mlp1 = mlpt[0:MID, :]
        for c in range(NCH):
            nc.tensor.matmul(out=mlp1[:, 0:1], lhsT=w1a[c], rhs=ssum[c],
                             start=(c == 0), stop=(c == 1))
        for c in range(NCH):
            nc.tensor.matmul(out=mlp1[:, 1:2], lhsT=w1m[c], rhs=smax[c],
                             start=(c == 0), stop=(c == 1))
        th1 = pstat.tile([MID, 2], F32, tag="th1")
        nc.scalar.activation(out=th1, in_=mlp1, func=Act.Tanh,
                             bias=mlpv[:, 1:2], scale=mlpv[:, 0:1])
        ca, cab, sfca = [], [], []
        for c in range(NCH):
            mlp2 = ps_mlp.tile([128, 2], F32, tag="mlp", name="mlp2")
            nc.tensor.matmul(out=mlp2[:, 0:1],
                             lhsT=w2t[:, c * 128:(c + 1) * 128],
                             rhs=th1[:, 0:1], start=True, stop=True)
            nc.tensor.matmul(out=mlp2[:, 1:2],
                             lhsT=w2t[:, c * 128:(c + 1) * 128],
                             rhs=th1[:, 1:2], start=True, stop=True)
            th2 = pstat.tile([128, 2], F32, tag="th2")
            nc.scalar.activation(out=th2, in_=mlp2, func=Act.Tanh,
                                 bias=chv[c][:, 1:2], scale=chv[c][:, 0:1])
            sab = pstat.tile([128, 1], F32, tag="sab")
            nc.vector.tensor_tensor(out=sab, in0=th2[:, 0:1],
                                    in1=th2[:, 1:2], op=Alu.add)
            cat = pstat.tile([128, 1], F32, tag="ca")
            nc.scalar.activation(out=cat, in_=sab, func=Act.Sigmoid)
            ca.append(cat)
            cb = pstat.tile([128, 1], BF16, tag="cab")
            nc.vector.tensor_copy(out=cb, in_=cat)
            cab.append(cb)
            sf = pstat.tile([128, 1], F32, tag="sfca")
            nc.vector.tensor_scalar(out=sf, in0=cat, scalar1=chv[c][:, 2:3],
                                    scalar2=None, op0=Alu.mult)
            sfca.append(sf)
        return ca, cab, sfca

    def emit_spatial(i, cab, ca):
        xb = [xbs[(i, c)] for c in range(NCH)]
        use_pmax = i < 2   # images 0,1: channel max on Pool; 2,3: PE+DVE

        if not use_pmax:
            diag = []
            for c in range(NCH):
                dg = pdiag.tile([128, 128], BF16, tag="diag")
                nc.gpsimd.affine_select(
                    out=dg, in_=cab[c].broadcast_to((128, 128)),
                    pattern=[[-1, 128]], compare_op=Alu.is_equal,
                    fill=0.0, base=0, channel_multiplier=1,
                )
                diag.append(dg)
        else:
            # products r_c = ca_c*x_c, pairwise max, partition-reduce on Pool
            r = []
            for c in range(NCH):
                rt = pr.tile([128, HW], BF16, tag="r")
                nc.vector.tensor_scalar(out=rt, in0=xb[c], scalar1=ca[c],
                                        scalar2=None, op0=Alu.mult)
                r.append(rt)
            rA = pr.tile([128, HW], BF16, tag="r")
            nc.vector.tensor_tensor(out=rA, in0=r[0], in1=r[1], op=Alu.max)
            rmax = pbig.tile([128, HW], BF16, tag="rmax", bufs=2)
            nc.gpsimd.partition_all_reduce(rmax, rA, channels=128,
                                           reduce_op=bass_isa.ReduceOp.max)

        # channel sum of ca*x -> psum pieces at partition bases {0,32,64}
        avgt = []
        for t in range(3):
            at = ps_avg.tile([65, APIECE], F32, tag="avg", name=f"avg{t}")
            avgt.append(at)
            for pp in range(3 if t < 2 else 1):
                p = t * 3 + pp
                sl = slice(p * APIECE, (p + 1) * APIECE)
                for c in range(NCH):
                    nc.tensor.matmul(out=at[pp * 32:pp * 32 + 1, :],
                                     lhsT=cab[c], rhs=xb[c][:, sl],
                                     start=(c == 0), stop=(c == 1))
        avgS = []
        for t in range(3):
            aS = psmall.tile([65, APIECE], BF16, tag=f"avgS{t}",
                             name=f"avgS{t}")
            nc.scalar.activation(out=aS, in_=avgt[t], func=Act.Copy)
            avgS.append(aS)

        if not use_pmax:
            # channel max of ca*x: transpose blocks via diag matmul + reduce
            mx = psmall.tile([GP, NGRP], F32, tag="mx")
            for s in range(NSUP):
                sup = ps_xt.tile([GP, SUPG, C], F32, tag="sup")
                for gg in range(SUPG):
                    g = s * SUPG + gg
                    for c in range(NCH):
                        nc.tensor.matmul(
                            out=sup[:, gg, c * 128:(c + 1) * 128],
                            lhsT=xb[c][:, g * GP:(g + 1) * GP], rhs=diag[c],
                            start=True, stop=True,
                        )
                nc.vector.tensor_reduce(out=mx[:, s * SUPG:(s + 1) * SUPG],
                                        in_=sup, axis=AxX, op=Alu.max)
            mxT_p = ps_sm.tile([56, APIECE], F32, tag="sm", name="mxTp")
            nc.tensor.transpose(out=mxT_p[0:NGRP, 0:GP], in_=mx,
                                identity=identF[0:GP, 0:GP])
            mxT = psmall.tile([NGRP, GP], BF16, tag="mxT")
            nc.scalar.activation(out=mxT, in_=mxT_p[0:NGRP, 0:GP],
                                 func=Act.Copy)

        # reflect-padded conv input S = [avg(58) ; max(58)]
        S = psmall.tile([116, 58], BF16, tag="S")
        for p in range(NAPIECE):
            nc.sync.dma_start(
                out=S[1 + 8 * p:9 + 8 * p, 1:57],
                in_=avgS[p // 3][(p % 3) * 32:(p % 3) * 32 + 1, :])
        nc.sync.dma_start(out=S[0:1, 1:57], in_=avgS[0][0:1, 56:112])
        nc.sync.dma_start(out=S[57:58, 1:57], in_=avgS[2][0:1, 336:392])
        if use_pmax:
            nc.sync.dma_start(out=S[59:115, 1:57], in_=rmax[0:1, 0:HW])
            nc.sync.dma_start(out=S[58:59, 1:57], in_=rmax[0:1, 56:112])
            nc.sync.dma_start(out=S[115:116, 1:57],
                              in_=rmax[0:1, 3024:3080])
        else:
            nc.sync.dma_start(out=S[59:115, 1:57], in_=mxT)
            nc.sync.dma_start(out=S[58:59, 1:57], in_=mxT[0:1, 56:112])
            nc.sync.dma_start(out=S[115:116, 1:57], in_=mxT[27:28, 0:56])
        nc.scalar.activation(out=S[:, 0:1], in_=S[:, 2:3], func=Act.Copy)
        nc.scalar.activation(out=S[:, 57:58], in_=S[:, 55:56], func=Act.Copy)

        # 3x3 conv as 3 banded matmuls + sigmoid -> sa
        conv = ps_sm.tile([56, APIECE], F32, tag="sm", name="conv")
        for dx in range(3):
            nc.tensor.matmul(out=conv[:, 0:56],
                             lhsT=bmat[:, dx * 56:(dx + 1) * 56],
                             rhs=S[:, dx:dx + 56],
                             start=(dx == 0), stop=(dx == 2))
        sa56 = psmall.tile([56, 56], BF16, tag="sa56")
        nc.scalar.activation(out=sa56, in_=conv[:, 0:56], func=Act.Sigmoid,
                             bias=cst56, scale=1.0)
        # bounce sa through DRAM, then broadcast to 128 partitions
        nc.sync.dma_start(out=scr_d[i:i + 1, :], in_=sa56)
        saB = pbig.tile([128, HW], BF16, tag="saB", bufs=2)
        nc.sync.dma_start(out=saB,
                          in_=scr_d[i:i + 1, :].to_broadcast((128, HW)))
        return saB

    # ---- software-pipelined front-end: pools staged two images ahead ----
    stats = {}
    for i in range(BLOC):
        stats[i] = ([pstat.tile([128, 1], F32, tag=f"ssum{c}{i % 2}",
                                name=f"ssum{c}_{i}") for c in range(NCH)],
                    [pstat.tile([128, 1], F32, tag=f"smax{c}{i % 2}",
                                name=f"smax{c}_{i}") for c in range(NCH)])

    def emit_finals(i):
        prods = {}
        for c in range(NCH):
            Mt = pbig.tile([128, HW], BF16, tag="M", bufs=3)
            nc.vector.tensor_scalar(out=Mt, in0=saB_all[i],
                                    scalar1=sfca_all[i][c],
                                    scalar2=chv[c][:, 2:3],
                                    op0=Alu.mult, op1=Alu.add)
            prod = pbig.tile([128, HW], BF16, tag="prod", bufs=4)
            nc.vector.tensor_tensor(out=prod, in0=xbs[(i, c)], in1=Mt,
                                    op=Alu.mult)
            prods[c] = prod
        for c in range(NCH):
            prod = prods[c]
            if i == BLOC - 1:
                nc.vector.tensor_scalar(out=prod, in0=prod,
                                        scalar1=chv[c][:, 3:4], scalar2=0.0,
                                        op0=Alu.add, op1=Alu.max)
            else:
                nc.scalar.activation(out=prod, in_=prod, func=Act.Relu,
                                     bias=chv[c][:, 3:4], scale=1.0)
            nc.sync.dma_start(
                out=y_d[i, c * 128:(c + 1) * 128, :], in_=prod)

    saB_all, sfca_all = {}, {}
    emit_pools(0, *stats[0])
    emit_pools(1, *stats[1])
    for i in range(BLOC):
        if i + 2 < BLOC:
            emit_pools(i + 2, *stats[i + 2])
        ca, cab, sfca = emit_mlp(*stats[i])
        sfca_all[i] = sfca
        saB_all[i] = emit_spatial(i, cab, ca)
    for i in range(BLOC):
        emit_finals(i)


# ---------------------------------------------------------------------------
# host-side parameter folding
# ---------------------------------------------------------------------------

def _fold_params(inp):
    f = lambda a: np.asarray(a, dtype=np.float32)
    import ml_dtypes

    s1 = f(inp["bn1_g"]) / np.sqrt(f(inp["bn1_v"]) + 1e-5)
    b1 = f(inp["bn1_b"]) - f(inp["bn1_m"]) * s1
    s2 = f(inp["bn2_g"]) / np.sqrt(f(inp["bn2_v"]) + 1e-5)
    b2 = f(inp["bn2_b"]) - f(inp["bn2_m"]) * s2
    sf = f(inp["fbn_g"]) / np.sqrt(f(inp["fbn_v"]) + 1e-5)
    bfb = f(inp["fbn_b"]) - f(inp["fbn_m"]) * sf

    w1 = f(inp["w1"])                      # [MID, C]
    w2 = f(inp["w2"])                      # [C, MID]
    w1t_avg = np.ascontiguousarray((w1 / HW).T)     # [C, MID]
    w1t_max = np.ascontiguousarray(w1.T)            # [C, MID]
    w2t = np.ascontiguousarray(w2.T)                # [MID, C]
    mlp_vec = np.stack([s1, b1], axis=1)            # [MID, 2]
    ch_vec = np.stack([s2, b2, sf, bfb], axis=1)    # [C, 4]

    # spatial conv folding: two BNs + conv bias + channel-mean divisor
    a1 = f(inp["sbn1_g"])[0] / np.sqrt(f(inp["sbn1_v"])[0] + 1e-3)
    c1 = f(inp["sbn1_b"])[0] - f(inp["sbn1_m"])[0] * a1
    a2 = f(inp["sbn2_g"])[0] / np.sqrt(f(inp["sbn2_v"])[0] + 1e-5)
    c2 = f(inp["sbn2_b"])[0] - f(inp["sbn2_m"])[0] * a2
    amul = a1 * a2
    cst = a2 * (a1 * f(inp["sconv_b"])[0] + c1) + c2

    wsp = f(inp["sconv_w"])[0]             # [2, 3, 3]
    w_eff = np.stack([wsp[0] * amul / C, wsp[1] * amul])  # [2(ic), 3(dy), 3(dx)]

    bmat = np.zeros((116, 168), np.float32)   # [ic*58+yp, dx*56+y]
    for ic in range(2):
        for dx in range(3):
            for y in range(56):
                for dy in range(3):
                    bmat[ic * 58 + y + dy, dx * 56 + y] = w_eff[ic, dy, dx]

    return {
        "w1t_avg": w1t_avg, "w1t_max": w1t_max, "w2t": w2t,
        "mlp_vec": np.ascontiguousarray(mlp_vec),
        "ch_vec": np.ascontiguousarray(ch_vec),
        "bmat": bmat.astype(ml_dtypes.bfloat16),
        "conv_cst": np.full((1, 1), cst, np.float32),
    }


_NC_CACHE = {}


def _get_program():
    if "nc" not in _NC_CACHE:
        _NC_CACHE["nc"] = _build_program()
    return _NC_CACHE["nc"]


def make_in_maps(inputs):
    import ml_dtypes
    params = _fold_params(inputs)
    x = np.asarray(inputs["x"], dtype=np.float32).reshape(B, C, HW)
    xb = x.astype(ml_dtypes.bfloat16)
    return [{"x_shard": np.ascontiguousarray(xb[core * BLOC:(core + 1) * BLOC]),
             **params} for core in range(NCORES)]


def kernel(**inputs) -> np.ndarray:
    nc = _get_program()
    in_maps = make_in_maps(inputs)
    res = bass_utils.run_bass_kernel_spmd(nc, in_maps, core_ids=list(range(NCORES)))
    out = np.concatenate([r["y_shard"].astype(np.float32) for r in res.results],
                         axis=0)
    return out.reshape(B, C, H, W)
